# revision 1
# baseline (speedup 1.0000x reference)
"""Trainium2 Bass kernel for nn_Block_42460046688864 (dense transformer block).

Reference math (B=2, T=2048, C=2048, H=16, HD=128):
    n1  = rmsnorm(x) * norm1_w
    qkv = n1 @ attn_w.T ; q,k,v per head ; q,k = rope(q,k) ; phi = elu(.)+1
    w   = (phi_q . phi_k) * scale * tril ; w /= sum(w) ; y = w @ v
    h   = y @ proj_w.T ; x2 = x + h
    ffn = gelu(rmsnorm(x2)*norm2_w @ fc_w.T) @ mlp_proj_w.T ; out = x2 + ffn

Distribution (8 NeuronCores, one NEFF, sequence-parallel Megatron):
  - rows (b*T+t, 4096 total) sharded 512/core for norms/residuals/output
  - attention head-sharded (2 heads/core) after an AllGather of n1^T
  - proj/mlp_proj row-parallel with ReduceScatter of partial sums
  - fc column-parallel (1024 hidden/core) after an AllGather of n2^T
  Attention itself is computed as *chunked linear attention*: the causal
  mask is exactly tril and elu+1 is positive, so sum-normalized masked
  scores equal prefix-state linear attention (the 1/sqrt(HD) scale and
  the 1e-8 epsilon cancel to ~1e-9 relative).

Notes:
  - norm weights are folded into attn_w / fc_w on the host (exact algebra).
  - matmul operands are bf16 (fp32 PSUM accumulation); norms, rope, elu,
    residuals and collective partial sums stay fp32.
  - TileContext's tail drain is patched to split its semaphore waits:
    this walrus build rejects >2 sync waits on one TPB_CTRL instruction.
"""

from contextlib import ExitStack

import numpy as np
import ml_dtypes

import concourse.bass as bass
import concourse.mybir as mybir
import concourse.tile as tile
from concourse.bass_utils import run_bass_kernel_spmd
from concourse.masks import make_identity
from bass_rust import ScopedClock

F32 = mybir.dt.float32
BF16 = mybir.dt.bfloat16
AF = mybir.ActivationFunctionType

N_CORES = 8
B, T, C, H, HD = 2, 2048, 2048, 16, 128
R = B * T                 # 4096 flattened rows (b-major)
R_LOC = R // N_CORES      # 512 rows per core
H_LOC = H // N_CORES      # 2 heads per core
F_LOC = (4 * C) // N_CORES  # 1024 mlp hidden per core
P = 128
EPS_NORM = 1e-5
N_RT = R_LOC // P         # 4 local row tiles
N_KC = C // P             # 16 contraction tiles over C
N_NB = R // 512           # 8 column blocks over flattened rows
N_CH = T // P             # 16 causal chunks per sequence


_MAX_WAITS = 1  # this walrus build rejects multi-wait instructions


def _split_excess_waits(nc):
    """Move excess semaphore waits onto same-engine NoOps ahead of the op."""
    for fn in nc.m.functions:
        for bb in fn.blocks:
            insts = list(bb.instructions)
            out = []
            for ins in insts:
                si = getattr(ins, "sync_info", None)
                waits = list(si.on_wait) if si and si.on_wait else []
                sem_waits = [w for w in waits if w.sync_type == "semaphore"]
                if len(sem_waits) > _MAX_WAITS:
                    keep = [w for w in waits if w.sync_type != "semaphore"]
                    keep += sem_waits[: _MAX_WAITS - 1] if _MAX_WAITS > 1 else []
                    extra = sem_waits[_MAX_WAITS - 1:] if _MAX_WAITS > 1 else sem_waits
                    for j in range(0, len(extra), _MAX_WAITS):
                        chunk = extra[j:j + _MAX_WAITS]
                        nop = mybir.InstNoOp(
                            name=nc.get_next_instruction_name(), ins=[], outs=[]
                        )
                        nop.engine = ins.engine
                        nop.sync_info = mybir.SyncInfo(on_wait=chunk, on_update=[])
                        out.append(nop)
                    si.on_wait[:] = keep
                out.append(ins)
            if len(out) != len(insts):
                bb.instructions[:] = out


class _TC(tile.TileContext):
    """TileContext whose tail drain splits sem waits one-per-NOP."""

    def schedule_and_allocate(self):
        ret = super().schedule_and_allocate()
        _split_excess_waits(self.nc)
        return ret

    def _drain_and_barrier(self, tick_clock, wait_clock):
        probe = self.nc.sync.nop(nofuse=True, hint="drain_waits")
        wait_clock.add_sem_waits(
            probe.ins, ScopedClock({None: tick_clock.global_clock})
        )
        si = probe.ins.sync_info
        waits = list(si.on_wait) if si and si.on_wait else []
        if len(waits) > 1:
            si.on_wait[:] = waits[:1]
            for w in waits[1:]:
                extra = self.nc.sync.nop(nofuse=True, hint="drain_waits")
                extra.ins.sync_info = mybir.SyncInfo(on_wait=[w], on_update=[])
        self.nc.sync.drain()
        self.nc.all_engine_barrier()
        popped = self.nc._tile_sem_poison_stack.pop()
        assert popped is self._sem_poison
        self.nc.clear_and_free_semaphores(list(self.sems.allocated().values()))
        self.nc.all_engine_barrier()


from contextlib import contextmanager


@contextmanager
def _low_priority(tc, offset=50000):
    tc.cur_priority += offset
    try:
        yield
    finally:
        tc.cur_priority -= offset


def _rmsnorm_transpose(nc, tc, pools, src_tiles, dstT_dram, ident_f32, eps_t):
    """rmsnorm rows of 4x[128,C] fp32 tiles -> bf16 transposed [C, 512] DRAM."""
    sq_pool, st_pool, n_pool, trp_pool, trc_pool = pools
    for i in range(N_RT):
        x_t = src_tiles[i]
        sq = sq_pool.tile([P, C], F32, name=f"sq{i}", tag="sq")
        ss = st_pool.tile([P, 1], F32, name=f"ss{i}", tag="ss")
        nc.scalar.activation(sq[:], x_t[:], AF.Square, accum_out=ss[:])
        rms = st_pool.tile([P, 1], F32, name=f"rms{i}", tag="rms")
        nc.scalar.activation(rms[:], ss[:], AF.Sqrt, bias=eps_t[:], scale=1.0 / C)
        inv = st_pool.tile([P, 1], F32, name=f"inv{i}", tag="inv")
        nc.vector.reciprocal(inv[:], rms[:])
        n_t = n_pool.tile([P, C], F32, name=f"n{i}", tag="n")
        nc.vector.tensor_scalar_mul(n_t[:], x_t[:], inv[:])
        for j in range(N_KC):
            ps = trp_pool.tile([P, P], F32, name=f"trp{i}_{j}", tag="trp")
            nc.tensor.transpose(ps[:], n_t[:, j * P:(j + 1) * P], ident_f32[:])
            cp = trc_pool.tile([P, P], BF16, name=f"trc{i}_{j}", tag="trc")
            nc.scalar.copy(cp[:], ps[:])
            nc.sync.dma_start(
                out=dstT_dram[j * P:(j + 1) * P, i * P:(i + 1) * P], in_=cp[:]
            )


def build_nc():
    nc = bass.Bass(target_bir_lowering=False)

    x_loc = nc.declare_dram_parameter("x_loc", [R_LOC, C], F32, isOutput=False)
    cosT = nc.declare_dram_parameter("cosT", [HD // 2, R], F32, isOutput=False)
    sinT = nc.declare_dram_parameter("sinT", [HD // 2, R], F32, isOutput=False)
    maskT = nc.declare_dram_parameter("maskT", [P, P], F32, isOutput=False)
    attn_wT = nc.declare_dram_parameter("attn_wT", [C, 3 * HD * H_LOC], BF16, isOutput=False)
    projwT = nc.declare_dram_parameter("projwT", [HD * H_LOC, C], BF16, isOutput=False)
    fcwT = nc.declare_dram_parameter("fcwT", [C, F_LOC], BF16, isOutput=False)
    mlpw = nc.declare_dram_parameter("mlpw", [F_LOC, C], BF16, isOutput=False)
    out_loc = nc.declare_dram_parameter("out_loc", [R_LOC, C], F32, isOutput=True)

    n1T_loc = nc.dram_tensor("n1T_loc", [C, R_LOC], BF16)
    n1T_all = nc.dram_tensor("n1T_all", [N_CORES, C, R_LOC], BF16, addr_space="Shared")
    h_part = nc.dram_tensor("h_part", [R, C], BF16)
    h_loc = nc.dram_tensor("h_loc", [R_LOC, C], BF16)
    n2T_loc = nc.dram_tensor("n2T_loc", [C, R_LOC], BF16)
    n2T_all = nc.dram_tensor("n2T_all", [N_CORES, C, R_LOC], BF16, addr_space="Shared")
    ffn_part = nc.dram_tensor("ffn_part", [R, C], BF16)
    ffn_loc = nc.dram_tensor("ffn_loc", [R_LOC, C], BF16)

    groups = [list(range(N_CORES))]

    with _TC(nc) as tc:
        with (
            tc.tile_pool(name="const", bufs=1) as const,
            tc.tile_pool(name="yT", bufs=1) as yT_pool,
        ):
            ident_f32 = const.tile([P, P], F32)
            make_identity(nc, ident_f32)
            ident_bf = const.tile([P, P], BF16)
            make_identity(nc, ident_bf)
            mask_sb = const.tile([P, P], F32)
            nc.sync.dma_start(out=mask_sb[:], in_=maskT[:, :])
            eps_t = const.tile([P, 1], F32)
            nc.vector.memset(eps_t[:], EPS_NORM)

            # per-t-chunk tiles so proj deps are precise (proj overlaps attention)
            yT = [
                [yT_pool.tile([P, P], BF16, name=f"yT{h}_{m}") for m in range(R // P)]
                for h in range(H_LOC)
            ]

            # ---- phase 0: rmsnorm(x_loc) -> n1T_loc; AllGather -> n1T_all
            with (
                tc.tile_pool(name="p0x", bufs=2) as p0x,
                tc.tile_pool(name="p0sq", bufs=2) as p0sq,
                tc.tile_pool(name="p0st", bufs=8) as p0st,
                tc.tile_pool(name="p0n", bufs=4) as p0n,
                tc.tile_pool(name="p0trp", bufs=4, space="PSUM") as p0trp,
                tc.tile_pool(name="p0trc", bufs=8) as p0trc,
            ):
                x_tiles = []
                for i in range(N_RT):
                    x_t = p0x.tile([P, C], F32, name=f"x{i}", tag=f"x{i}")
                    nc.sync.dma_start(out=x_t[:], in_=x_loc[i * P:(i + 1) * P, :])
                    x_tiles.append(x_t)
                _rmsnorm_transpose(
                    nc, tc, (p0sq, p0st, p0n, p0trp, p0trc), x_tiles, n1T_loc, ident_f32, eps_t
                )
                nc.gpsimd.collective_compute(
                    "AllGather",
                    mybir.AluOpType.bypass,
                    ins=[n1T_loc.ap().opt()],
                    outs=[n1T_all.ap().opt()],
                    replica_groups=groups,
                )

            # ---- phase 1: qkv^T for 2 local heads + rope + elu+1 -> Q/K/V
            # resident [128, 4096] bf16 per (head, comp)
            with tc.tile_pool(name="qkvres", bufs=1) as qkv_pool:
                qres = [qkv_pool.tile([P, R], BF16, name=f"q{h}") for h in range(H_LOC)]
                kres = [qkv_pool.tile([P, R], BF16, name=f"k{h}") for h in range(H_LOC)]
                vres = [qkv_pool.tile([P, R], BF16, name=f"v{h}") for h in range(H_LOC)]

                with (
                    tc.tile_pool(name="p1w", bufs=1) as p1w,
                    tc.tile_pool(name="p1cs", bufs=1) as p1cs,
                    tc.tile_pool(name="p1rhs", bufs=18) as p1rhs,
                    tc.tile_pool(name="p1ps", bufs=4, space="PSUM") as p1ps,
                    tc.tile_pool(name="p1rp", bufs=4) as p1rp,
                ):
                    cos_sb = p1cs.tile([HD // 2, R], F32, name="cos_sb")
                    sin_sb = p1cs.tile([HD // 2, R], F32, name="sin_sb")
                    nc.sync.dma_start(out=cos_sb[:], in_=cosT[:, :])
                    nc.sync.dma_start(out=sin_sb[:], in_=sinT[:, :])
                    aw = []
                    for k in range(N_KC):
                        w_t = p1w.tile([P, 3 * HD * H_LOC], BF16, name=f"aw{k}", tag=f"aw{k}")
                        nc.sync.dma_start(
                            out=w_t[:], in_=attn_wT[k * P:(k + 1) * P, :]
                        )
                        aw.append(w_t)

                    for nb in range(N_NB):
                        rhs = []
                        for k in range(N_KC):
                            r_t = p1rhs.tile([P, 512], BF16, name=f"n1r{nb}_{k}", tag="n1r")
                            nc.sync.dma_start(
                                out=r_t[:],
                                in_=n1T_all[nb, k * P:(k + 1) * P, :],
                            )
                            rhs.append(r_t)
                        ncol = slice(nb * 512, (nb + 1) * 512)
                        for h in range(H_LOC):
                            for comp in range(3):
                                j = h * 3 + comp
                                ps = p1ps.tile([P, 512], F32, name=f"qkvp{nb}_{j}", tag="qkvp")
                                for k in range(N_KC):
                                    nc.tensor.matmul(
                                        ps[:],
                                        aw[k][:, j * P:(j + 1) * P],
                                        rhs[k][:],
                                        start=(k == 0),
                                        stop=(k == N_KC - 1),
                                    )
                                if comp == 2:
                                    nc.scalar.copy(vres[h][:, ncol], ps[:])
                                else:
                                    dst = qres[h] if comp == 0 else kres[h]
                                    HF = HD // 2
                                    ro = p1rp.tile([P, 512], F32, name=f"ro{nb}_{j}", tag="ro")
                                    s1 = p1rp.tile([HF, 512], F32, name=f"s1{nb}_{j}", tag="s1")
                                    s2 = p1rp.tile([HF, 512], F32, name=f"s2{nb}_{j}", tag="s2")
                                    # rope: out[0:64] = a1*cos - a2*sin ; out[64:128] = a1*sin + a2*cos
                                    nc.vector.tensor_mul(s1[:], ps[0:HF, :], cos_sb[:, ncol])
                                    nc.vector.tensor_mul(s2[:], ps[HF:P, :], sin_sb[:, ncol])
                                    nc.vector.tensor_sub(ro[0:HF, :], s1[:], s2[:])
                                    nc.vector.tensor_mul(s1[:], ps[0:HF, :], sin_sb[:, ncol])
                                    nc.vector.tensor_mul(s2[:], ps[HF:P, :], cos_sb[:, ncol])
                                    nc.vector.tensor_add(ro[HF:P, :], s1[:], s2[:])
                                    # phi = elu(ro)+1 = relu(ro) + exp(ro - relu(ro))
                                    rl = p1rp.tile([P, 512], F32, name=f"rl{nb}_{j}", tag="rl")
                                    nc.scalar.activation(rl[:], ro[:], AF.Relu)
                                    dmin = p1rp.tile([P, 512], F32, name=f"dm{nb}_{j}", tag="dm")
                                    nc.vector.tensor_sub(dmin[:], ro[:], rl[:])
                                    ex = p1rp.tile([P, 512], F32, name=f"ex{nb}_{j}", tag="ex")
                                    nc.scalar.activation(ex[:], dmin[:], AF.Exp)
                                    nc.vector.tensor_add(dst[:, ncol], rl[:], ex[:])

                # ---- phase 2: chunked linear attention per (head, b)
                with (
                    tc.tile_pool(name="p2st", bufs=1) as p2st,
                    tc.tile_pool(name="p2sbf", bufs=3) as p2sbf,
                    tc.tile_pool(name="p2sb", bufs=8) as p2sb,
                    tc.tile_pool(name="p2psA", bufs=3, space="PSUM") as p2psA,
                    tc.tile_pool(name="p2psY", bufs=3, space="PSUM") as p2psY,
                    tc.tile_pool(name="p2psS", bufs=2, space="PSUM") as p2psS,
                ):
                    s_sb_d = {}
                    s_bf_d = {}
                    for h in range(H_LOC):
                        for b in range(B):
                            s_sb = p2st.tile([P, HD + 1], F32, name=f"S{h}_{b}")
                            nc.vector.memset(s_sb[:], 0.0)
                            s_bf = p2sbf.tile([P, HD + 1], BF16, name=f"Sb{h}_{b}_init", tag=f"sbf{h}{b}")
                            nc.vector.memset(s_bf[:], 0.0)
                            s_sb_d[(h, b)] = s_sb
                            s_bf_d[(h, b)] = s_bf
                    for i in range(N_CH):
                        for h in range(H_LOC):
                            for b in range(B):
                                s_sb = s_sb_d[(h, b)]
                                s_bf = s_bf_d[(h, b)]
                                t0 = b * T + i * P
                                tcol = slice(t0, t0 + P)
                                # A^T[s,t] = sum_d K^T[d,s] Q^T[d,t]
                                a_ps = p2psA.tile([P, P], F32, name=f"A{h}{b}{i}", tag="A")
                                nc.tensor.matmul(
                                    a_ps[:], kres[h][:, tcol], qres[h][:, tcol],
                                    start=True, stop=True,
                                )
                                am = p2sb.tile([P, P], BF16, name=f"Am{h}{b}{i}", tag="Am")
                                nc.vector.tensor_mul(am[:], a_ps[:], mask_sb[:])
                                # V' = [V_chunk | 1], K_chunk row-major via DMA transpose
                                vp = p2sb.tile([P, HD + 1], BF16, name=f"Vp{h}{b}{i}", tag="Vp")
                                nc.vector.memset(vp[:, HD:HD + 1], 1.0)
                                nc.sync.dma_start_transpose(vp[:, 0:HD], vres[h][:, tcol])
                                kp = p2sb.tile([P, P], BF16, name=f"Kp{h}{b}{i}", tag="Kp")
                                nc.sync.dma_start_transpose(kp[:], kres[h][:, tcol])
                                # Y = Q_chunk @ S' + Am^T @ V'  (last col = denominator)
                                y_ps = p2psY.tile([P, HD + 1], F32, name=f"Y{h}{b}{i}", tag="Y")
                                nc.tensor.matmul(
                                    y_ps[:], qres[h][:, tcol], s_bf[:],
                                    start=True, stop=False,
                                )
                                nc.tensor.matmul(
                                    y_ps[:], am[:], vp[:], start=False, stop=True
                                )
                                # state += K_chunk^T-outer-V'
                                sd_ps = p2psS.tile([P, HD + 1], F32, name=f"Sd{h}{b}{i}", tag="Sd")
                                nc.tensor.matmul(
                                    sd_ps[:], kp[:], vp[:], start=True, stop=True
                                )
                                nc.vector.tensor_add(s_sb[:], s_sb[:], sd_ps[:])
                                s_bf = p2sbf.tile([P, HD + 1], BF16, name=f"Sb{h}_{b}_{i}", tag=f"sbf{h}{b}")
                                nc.scalar.copy(s_bf[:], s_sb[:])
                                s_bf_d[(h, b)] = s_bf
                                # y = num/den ; write y^T
                                rec = p2sb.tile([P, 1], F32, name=f"rec{h}{b}{i}", tag="rec")
                                nc.vector.reciprocal(rec[:], y_ps[:, HD:HD + 1])
                                y_sb = p2sb.tile([P, HD], BF16, name=f"y{h}{b}{i}", tag="y")
                                nc.vector.tensor_scalar_mul(y_sb[:], y_ps[:, 0:HD], rec[:])
                                nc.sync.dma_start_transpose(
                                    yT[h][b * N_CH + i][:], y_sb[:]
                                )

            # ---- phase 3: h_part = y^T.T @ projwT (row-parallel partial)
            with (
                tc.tile_pool(name="p3w", bufs=1) as p3w,
                tc.tile_pool(name="p3ps", bufs=4, space="PSUM") as p3ps,
                tc.tile_pool(name="p3ev", bufs=8) as p3ev,
            ):
                pw = []
                for kd in range(H_LOC):
                    w_t = p3w.tile([P, C], BF16, name=f"pw{kd}", tag=f"pw{kd}")
                    nc.sync.dma_start(out=w_t[:], in_=projwT[kd * P:(kd + 1) * P, :])
                    pw.append(w_t)
                for mt in range(R // P):
                    mcol = slice(mt * P, (mt + 1) * P)
                    for ont in range(C // 512):
                        ps = p3ps.tile([P, 512], F32, name=f"hp{mt}_{ont}", tag="hp")
                        for kd in range(H_LOC):
                            nc.tensor.matmul(
                                ps[:],
                                yT[kd][mt][:],
                                pw[kd][:, ont * 512:(ont + 1) * 512],
                                start=(kd == 0),
                                stop=(kd == H_LOC - 1),
                            )
                        ev = p3ev.tile([P, 512], BF16, name=f"he{mt}_{ont}", tag="he")
                        nc.scalar.copy(ev[:], ps[:])
                        nc.sync.dma_start(
                            out=h_part[mt * P:(mt + 1) * P, ont * 512:(ont + 1) * 512],
                            in_=ev[:],
                        )
                nc.gpsimd.collective_compute(
                    "ReduceScatter",
                    mybir.AluOpType.add,
                    ins=[h_part.ap().opt()],
                    outs=[h_loc.ap().opt()],
                    replica_groups=groups,
                )

            # ---- phase 4: x2 = x + h (own rows); rmsnorm2 -> n2T; AllGather
            x2_ctx = ExitStack()
            x2_pool = x2_ctx.enter_context(tc.tile_pool(name="x2res", bufs=1))
            x2_res = [x2_pool.tile([P, C], F32, name=f"x2_{i}") for i in range(N_RT)]
            with (
                tc.tile_pool(name="p4h", bufs=4) as p4h,
                tc.tile_pool(name="p4sq", bufs=2) as p4sq,
                tc.tile_pool(name="p4st", bufs=8) as p4st,
                tc.tile_pool(name="p4n", bufs=4) as p4n,
                tc.tile_pool(name="p4trp", bufs=4, space="PSUM") as p4trp,
                tc.tile_pool(name="p4trc", bufs=8) as p4trc,
            ):
                for i in range(N_RT):
                    hb_t = p4h.tile([P, C], BF16, name=f"hb{i}", tag="hb")
                    nc.sync.dma_start(out=hb_t[:], in_=h_loc[i * P:(i + 1) * P, :])
                    h_t = p4h.tile([P, C], F32, name=f"h{i}", tag="h")
                    nc.scalar.copy(h_t[:], hb_t[:])
                    x_t = p4h.tile([P, C], F32, name=f"x4_{i}", tag="x4")
                    nc.sync.dma_start(out=x_t[:], in_=x_loc[i * P:(i + 1) * P, :])
                    nc.vector.tensor_add(x2_res[i][:], x_t[:], h_t[:])
                _rmsnorm_transpose(
                    nc, tc, (p4sq, p4st, p4n, p4trp, p4trc), x2_res, n2T_loc, ident_f32, eps_t
                )
                nc.gpsimd.collective_compute(
                    "AllGather",
                    mybir.AluOpType.bypass,
                    ins=[n2T_loc.ap().opt()],
                    outs=[n2T_all.ap().opt()],
                    replica_groups=groups,
                )

            # ---- phase 5: gT = gelu(fcwT.T @ n2T); ffn_part = gT.T @ mlpw
            with (
                tc.tile_pool(name="p5fw", bufs=1) as p5fw,
                tc.tile_pool(name="p5mw", bufs=1) as p5mw,
                tc.tile_pool(name="p5rhs", bufs=18) as p5rhs,
                tc.tile_pool(name="p5g", bufs=1) as p5g,
                tc.tile_pool(name="p5ps", bufs=3, space="PSUM") as p5ps,
                tc.tile_pool(name="p5ps2", bufs=3, space="PSUM") as p5ps2,
                tc.tile_pool(name="p5ev", bufs=4) as p5ev,
            ):
                fw = []
                for k in range(N_KC):
                    w_t = p5fw.tile([P, F_LOC], BF16, name=f"fw{k}", tag=f"fw{k}")
                    nc.sync.dma_start(out=w_t[:], in_=fcwT[k * P:(k + 1) * P, :])
                    fw.append(w_t)
                mw = []
                for k in range(F_LOC // P):
                    w_t = p5mw.tile([P, C], BF16, name=f"mw{k}", tag=f"mw{k}")
                    nc.sync.dma_start(out=w_t[:], in_=mlpw[k * P:(k + 1) * P, :])
                    mw.append(w_t)

                gk = [None] * (F_LOC // P)
                for nb in range(N_NB):
                    rhs = []
                    for k in range(N_KC):
                        r_t = p5rhs.tile([P, 512], BF16, name=f"n2r{nb}_{k}", tag="n2r")
                        nc.sync.dma_start(
                            out=r_t[:], in_=n2T_all[nb, k * P:(k + 1) * P, :]
                        )
                        rhs.append(r_t)
                    for mf in range(F_LOC // P):
                        ps = p5ps.tile([P, 512], F32, name=f"gp{nb}_{mf}", tag="gp")
                        for k in range(N_KC):
                            nc.tensor.matmul(
                                ps[:],
                                fw[k][:, mf * P:(mf + 1) * P],
                                rhs[k][:],
                                start=(k == 0),
                                stop=(k == N_KC - 1),
                            )
                        g_t = p5g.tile([P, 512], BF16, name=f"g{nb}_{mf}", tag=f"g{mf}", bufs=2)
                        nc.scalar.activation(g_t[:], ps[:], AF.Gelu)
                        gk[mf] = g_t
                    for mt in range(4):
                        mcol = slice(mt * P, (mt + 1) * P)
                        row0 = nb * 512 + mt * P
                        for ont in range(C // 512):
                            ps2 = p5ps2.tile([P, 512], F32, name=f"fp{nb}_{mt}_{ont}", tag="fp")
                            for kf in range(F_LOC // P):
                                nc.tensor.matmul(
                                    ps2[:],
                                    gk[kf][:, mcol],
                                    mw[kf][:, ont * 512:(ont + 1) * 512],
                                    start=(kf == 0),
                                    stop=(kf == F_LOC // P - 1),
                                )
                            ev = p5ev.tile([P, 512], BF16, name=f"fe{nb}_{mt}_{ont}", tag="fe")
                            nc.scalar.copy(ev[:], ps2[:])
                            nc.sync.dma_start(
                                out=ffn_part[row0:row0 + P, ont * 512:(ont + 1) * 512],
                                in_=ev[:],
                            )
                nc.gpsimd.collective_compute(
                    "ReduceScatter",
                    mybir.AluOpType.add,
                    ins=[ffn_part.ap().opt()],
                    outs=[ffn_loc.ap().opt()],
                    replica_groups=groups,
                )

            # ---- phase 6: out = x2 + ffn (own rows)
            with tc.tile_pool(name="p6", bufs=2) as p6:
                for i in range(N_RT):
                    fb_t = p6.tile([P, C], BF16, name=f"fb{i}", tag="fb")
                    nc.sync.dma_start(out=fb_t[:], in_=ffn_loc[i * P:(i + 1) * P, :])
                    f_t = p6.tile([P, C], F32, name=f"f{i}", tag="f")
                    nc.scalar.copy(f_t[:], fb_t[:])
                    o_t = p6.tile([P, C], F32, name=f"o{i}", tag="o")
                    nc.vector.tensor_add(o_t[:], x2_res[i][:], f_t[:])
                    nc.sync.dma_start(out=out_loc[i * P:(i + 1) * P, :], in_=o_t[:])
            x2_ctx.close()

    return nc


_NC_CACHE = None


def _get_nc():
    global _NC_CACHE
    if _NC_CACHE is None:
        _NC_CACHE = build_nc()
    return _NC_CACHE


def _prep_inputs(x, cos, sin, attention_bias, norm1_w, norm2_w, attn_w, proj_w,
                 fc_w, mlp_proj_w):
    bf = ml_dtypes.bfloat16
    xf = np.asarray(x, np.float32).reshape(R, C)
    cosT = np.ascontiguousarray(
        np.concatenate([np.asarray(cos, np.float32).T] * B, axis=1)
    )
    sinT = np.ascontiguousarray(
        np.concatenate([np.asarray(sin, np.float32).T] * B, axis=1)
    )
    # mask[s, t] = 1 iff s <= t  (transposed causal tril)
    maskT = np.triu(np.ones((P, P), np.float32))
    w1 = np.asarray(norm1_w, np.float32)
    w2 = np.asarray(norm2_w, np.float32)
    aw = np.asarray(attn_w, np.float32).reshape(H, 3, HD, C)
    pw = np.asarray(proj_w, np.float32)
    fw = np.asarray(fc_w, np.float32)
    mw = np.asarray(mlp_proj_w, np.float32)

    in_maps = []
    for c in range(N_CORES):
        aw_c = (aw[2 * c:2 * c + 2].reshape(3 * HD * H_LOC, C) * w1[None, :])
        fw_c = fw[F_LOC * c:F_LOC * (c + 1)] * w2[None, :]
        in_maps.append({
            "x_loc": np.ascontiguousarray(xf[R_LOC * c:R_LOC * (c + 1)]),
            "cosT": cosT,
            "sinT": sinT,
            "maskT": maskT,
            "attn_wT": np.ascontiguousarray(aw_c.T).astype(bf),
            "projwT": np.ascontiguousarray(
                pw[:, HD * H_LOC * c:HD * H_LOC * (c + 1)].T
            ).astype(bf),
            "fcwT": np.ascontiguousarray(fw_c.T).astype(bf),
            "mlpw": np.ascontiguousarray(
                mw[:, F_LOC * c:F_LOC * (c + 1)].T
            ).astype(bf),
        })
    return in_maps


def kernel(**inputs):
    nc = _get_nc()
    in_maps = _prep_inputs(**inputs)
    res = run_bass_kernel_spmd(nc, in_maps, list(range(N_CORES)))
    out = np.concatenate(
        [np.asarray(res.results[c]["out_loc"], np.float32) for c in range(N_CORES)],
        axis=0,
    )
    return out.reshape(B, T, C)



# revision 6
# speedup vs baseline: 1.6089x; 1.6089x over previous
"""Trainium2 Bass kernel for nn_Block_42460046688864 (dense transformer block).

Reference math (B=2, T=2048, C=2048, H=16, HD=128):
    n1  = rmsnorm(x) * norm1_w
    qkv = n1 @ attn_w.T ; q,k,v per head ; q,k = rope(q,k) ; phi = elu(.)+1
    w   = (phi_q . phi_k) * scale * tril ; w /= sum(w) ; y = w @ v
    h   = y @ proj_w.T ; x2 = x + h
    ffn = gelu(rmsnorm(x2)*norm2_w @ fc_w.T) @ mlp_proj_w.T ; out = x2 + ffn

Distribution (8 NeuronCores, one NEFF): pure data-parallel over rows.
Each core owns 512 consecutive flattened rows (b-major), computes the
whole block for them, and streams the full weights from HBM (~100MB,
overlapped with ~680us of bf16 matmul).  The causal sum-normalized
elu-kernel attention is computed as chunked linear attention (exactly
equal: the tril mask + positive feature map make masked sum-normalized
scores a prefix recursion; 1/sqrt(HD) and the 1e-8 eps cancel).  The
only cross-core exchange is each core's per-head prefix state
(phi_k^T @ [v|1], 16 x [128,129] bf16 = 528KB), AllGather'd within the
4-core group that shares a batch element, then prefix-masked per core.

Everything else is local: no activation AllGathers/ReduceScatters.

Notes:
  - norm weights are folded into attn_w / fc_w on the host (exact algebra).
  - matmul operands bf16 (fp32 PSUM accumulation); rope/elu elementwise
    runs in bf16 SBUF for the DVE fast modes; residuals stay fp32.
  - TileContext's tail drain is patched to split its semaphore waits:
    this walrus build rejects >2 sync waits on one TPB_CTRL instruction.
"""

from contextlib import ExitStack

import numpy as np
import ml_dtypes

import concourse.bass as bass
import concourse.mybir as mybir
import concourse.tile as tile
from concourse.bass_utils import run_bass_kernel_spmd
from concourse.masks import make_identity
from bass_rust import ScopedClock

F32 = mybir.dt.float32
BF16 = mybir.dt.bfloat16
AF = mybir.ActivationFunctionType

N_CORES = 8
GROUP = 4                  # cores per batch element
B, T, C, H, HD = 2, 2048, 2048, 16, 128
HF = HD // 2
R = B * T                  # 4096 flattened rows (b-major)
R_LOC = R // N_CORES       # 512 rows per core
P = 128
N_RT = R_LOC // P          # 4 local row tiles == 4 causal chunks
N_KC = C // P              # 16 contraction tiles over C
FD = 4 * C                 # 8192 mlp hidden
N_FT = FD // P             # 64 hidden tiles
SB = HD + 1                # state cols: [v | 1]
EPS_NORM = 1e-5

_MAX_WAITS = 1  # this walrus build rejects multi-wait instructions


def _split_excess_waits(nc):
    """Move excess semaphore waits onto same-engine NoOps ahead of the op."""
    for fn in nc.m.functions:
        for bb in fn.blocks:
            insts = list(bb.instructions)
            out = []
            for ins in insts:
                si = getattr(ins, "sync_info", None)
                waits = list(si.on_wait) if si and si.on_wait else []
                sem_waits = [w for w in waits if w.sync_type == "semaphore"]
                if len(sem_waits) > _MAX_WAITS:
                    keep = [w for w in waits if w.sync_type != "semaphore"]
                    keep += sem_waits[: _MAX_WAITS - 1] if _MAX_WAITS > 1 else []
                    extra = sem_waits[_MAX_WAITS - 1:] if _MAX_WAITS > 1 else sem_waits
                    for j in range(0, len(extra), _MAX_WAITS):
                        chunk = extra[j:j + _MAX_WAITS]
                        nop = mybir.InstNoOp(
                            name=nc.get_next_instruction_name(), ins=[], outs=[]
                        )
                        nop.engine = ins.engine
                        nop.sync_info = mybir.SyncInfo(on_wait=chunk, on_update=[])
                        out.append(nop)
                    si.on_wait[:] = keep
                out.append(ins)
            if len(out) != len(insts):
                bb.instructions[:] = out


class _TC(tile.TileContext):
    """TileContext whose tail drain splits sem waits one-per-NOP."""

    def schedule_and_allocate(self):
        ret = super().schedule_and_allocate()
        _split_excess_waits(self.nc)
        return ret

    def _drain_and_barrier(self, tick_clock, wait_clock):
        probe = self.nc.sync.nop(nofuse=True, hint="drain_waits")
        wait_clock.add_sem_waits(
            probe.ins, ScopedClock({None: tick_clock.global_clock})
        )
        si = probe.ins.sync_info
        waits = list(si.on_wait) if si and si.on_wait else []
        if len(waits) > 1:
            si.on_wait[:] = waits[:1]
            for w in waits[1:]:
                extra = self.nc.sync.nop(nofuse=True, hint="drain_waits")
                extra.ins.sync_info = mybir.SyncInfo(on_wait=[w], on_update=[])
        self.nc.sync.drain()
        self.nc.all_engine_barrier()
        popped = self.nc._tile_sem_poison_stack.pop()
        assert popped is self._sem_poison
        self.nc.clear_and_free_semaphores(list(self.sems.allocated().values()))
        self.nc.all_engine_barrier()


def build_nc():
    nc = bass.Bass(target_bir_lowering=False)

    x_loc = nc.declare_dram_parameter("x_loc", [R_LOC, C], F32, isOutput=False)
    cosT = nc.declare_dram_parameter("cosT", [HF, R_LOC], BF16, isOutput=False)
    sinT = nc.declare_dram_parameter("sinT", [HF, R_LOC], BF16, isOutput=False)
    maskT = nc.declare_dram_parameter("maskT", [P, P], F32, isOutput=False)
    pmaskp = nc.declare_dram_parameter("pmaskp", [P, GROUP], F32, isOutput=False)
    # attn weight, norm1 folded, transposed; column order [k(16h) | v(16h) | q(16h)]
    attn_wT = nc.declare_dram_parameter("attn_wT", [C, 3 * C], BF16, isOutput=False)
    projwT = nc.declare_dram_parameter("projwT", [C, C], BF16, isOutput=False)
    fcwT = nc.declare_dram_parameter("fcwT", [C, FD], BF16, isOutput=False)
    mlpw = nc.declare_dram_parameter("mlpw", [FD, C], BF16, isOutput=False)
    out_loc = nc.declare_dram_parameter("out_loc", [R_LOC, C], F32, isOutput=True)

    st_loc = nc.dram_tensor("st_loc", [P, H * SB], BF16)
    st_all = nc.dram_tensor("st_all", [GROUP, P, H * SB], BF16)
    groups = [list(range(GROUP)), list(range(GROUP, 2 * GROUP))]

    with _TC(nc) as tc:
        with ExitStack() as top:
            const = top.enter_context(tc.tile_pool(name="const", bufs=1))
            ident_bf = const.tile([P, P], BF16)
            make_identity(nc, ident_bf)
            mask_sb = const.tile([P, P], F32)
            nc.sync.dma_start(out=mask_sb[:], in_=maskT[:, :])
            pmask_sb = const.tile([P, GROUP], F32)
            nc.sync.dma_start(out=pmask_sb[:], in_=pmaskp[:, :])
            eps_t = const.tile([P, 1], F32)
            nc.vector.memset(eps_t[:], EPS_NORM)
            cos_sb = const.tile([HF, R_LOC], BF16)
            sin_sb = const.tile([HF, R_LOC], BF16)
            nc.sync.dma_start(out=cos_sb[:], in_=cosT[:, :])
            nc.sync.dma_start(out=sin_sb[:], in_=sinT[:, :])

            # -------- residents spanning phases A..D (yT) and A..C ---------
            yT_ctx = ExitStack()
            yT_pool = yT_ctx.enter_context(tc.tile_pool(name="yT", bufs=1))
            yT = [yT_pool.tile([P, R_LOC], BF16, name=f"yT{h}") for h in range(H)]

            bc_ctx = ExitStack()
            n1T_pool = bc_ctx.enter_context(tc.tile_pool(name="n1T", bufs=1))
            qkv_pool = bc_ctx.enter_context(tc.tile_pool(name="qkvT", bufs=1))
            vp_pool = bc_ctx.enter_context(tc.tile_pool(name="vp", bufs=1))
            e_pool = bc_ctx.enter_context(tc.tile_pool(name="estate", bufs=1))
            pfx_pool = bc_ctx.enter_context(tc.tile_pool(name="prefix", bufs=1))

            n1T = [n1T_pool.tile([P, R_LOC], BF16, name=f"n1T{c}") for c in range(N_KC)]
            kTt = [qkv_pool.tile([P, R_LOC], BF16, name=f"kT{h}") for h in range(H)]
            qTt = [qkv_pool.tile([P, R_LOC], BF16, name=f"qT{h}") for h in range(H)]
            vp = [
                [vp_pool.tile([P, SB], BF16, name=f"vp{h}_{i}") for i in range(N_RT)]
                for h in range(H)
            ]
            # bf16 exclusive local-state snapshots E_1..E_3 per head + f32 chain
            e_st = [
                [e_pool.tile([P, SB], BF16, name=f"e{h}_{i}") for i in range(3)]
                for h in range(H)
            ]
            e_run = [e_pool.tile([P, SB], F32, name=f"er{h}") for h in range(H)]

            # ---------------- phase A: load x, rmsnorm, transpose -> n1T ----
            a_ctx = ExitStack()
            xa_pool = a_ctx.enter_context(tc.tile_pool(name="xa", bufs=1))
            with (
                tc.tile_pool(name="a_st", bufs=1) as a_st,
                tc.tile_pool(name="a_nb", bufs=1) as a_nb,
                tc.tile_pool(name="a_ps", bufs=4, space="PSUM") as a_ps,
            ):
                for i in range(N_RT):
                    x_t = xa_pool.tile([P, C], F32, name=f"xa{i}")
                    nc.sync.dma_start(out=x_t[:], in_=x_loc[i * P:(i + 1) * P, :])
                    sq = a_nb.tile([P, C], F32, name=f"sq{i}", tag="sq", bufs=2)
                    ss = a_st.tile([P, 1], F32, name=f"ss{i}", tag="ss", bufs=2)
                    nc.scalar.activation(sq[:], x_t[:], AF.Square, accum_out=ss[:])
                    rms = a_st.tile([P, 1], F32, name=f"rms{i}", tag="rms", bufs=2)
                    nc.scalar.activation(rms[:], ss[:], AF.Sqrt, bias=eps_t[:], scale=1.0 / C)
                    inv = a_st.tile([P, 1], F32, name=f"inv{i}", tag="inv", bufs=2)
                    nc.vector.reciprocal(inv[:], rms[:])
                    nb = a_nb.tile([P, C], BF16, name=f"n1b{i}", tag="n1b", bufs=2)
                    nc.scalar.activation(nb[:], x_t[:], AF.Copy, scale=inv[:])
                    for j in range(N_KC):
                        ps = a_ps.tile([P, P], BF16, name=f"atr{i}_{j}", tag="atr")
                        nc.tensor.transpose(ps[:], nb[:, j * P:(j + 1) * P], ident_bf[:])
                        nc.scalar.copy(n1T[j][:, i * P:(i + 1) * P], ps[:])
            a_ctx.close()  # x tiles freed (x reloaded for the residual later)

            # ---------------- phase B: qkv + rope/elu + states + AllGather --
            def rope_elu(ps, dst, rp):
                """psum [P,512] (hd x t) -> dst bf16 [P,512] = elu(rope(.))+1."""
                qe = rp.tile([P, R_LOC], BF16, name="qe", tag="qe", bufs=3)
                nc.scalar.copy(qe[:], ps[:])
                ro = rp.tile([P, R_LOC], BF16, name="ro", tag="ro", bufs=2)
                s1 = rp.tile([HF, R_LOC], BF16, name="s1", tag="s1", bufs=2)
                s2 = rp.tile([HF, R_LOC], BF16, name="s2", tag="s2", bufs=2)
                nc.vector.tensor_mul(s1[:], qe[0:HF, :], cos_sb[:])
                nc.vector.tensor_mul(s2[:], qe[HF:P, :], sin_sb[:])
                nc.vector.tensor_sub(ro[0:HF, :], s1[:], s2[:])
                s3 = rp.tile([HF, R_LOC], BF16, name="s3", tag="s3", bufs=2)
                s4 = rp.tile([HF, R_LOC], BF16, name="s4", tag="s4", bufs=2)
                nc.vector.tensor_mul(s3[:], qe[0:HF, :], sin_sb[:])
                nc.vector.tensor_mul(s4[:], qe[HF:P, :], cos_sb[:])
                nc.vector.tensor_add(ro[HF:P, :], s3[:], s4[:])
                # phi = relu(ro) + exp(ro - relu(ro))
                rl = rp.tile([P, R_LOC], BF16, name="rl", tag="rl", bufs=2)
                nc.vector.tensor_scalar_max(rl[:], ro[:], 0.0)
                dm = rp.tile([P, R_LOC], BF16, name="dm", tag="dm", bufs=2)
                nc.vector.tensor_sub(dm[:], ro[:], rl[:])
                ex = rp.tile([P, R_LOC], BF16, name="ex", tag="ex", bufs=2)
                nc.scalar.activation(ex[:], dm[:], AF.Exp)
                nc.vector.tensor_add(dst[:], rl[:], ex[:])

            with (
                tc.tile_pool(name="b_aw", bufs=1) as b_aw,
                tc.tile_pool(name="b_rp", bufs=1) as b_rp,
                tc.tile_pool(name="b_vt", bufs=4) as b_vt,
                tc.tile_pool(name="b_kp", bufs=1) as b_kp,
                tc.tile_pool(name="b_stb", bufs=4) as b_stb,
                tc.tile_pool(name="b_ps", bufs=3, space="PSUM") as b_ps,
                tc.tile_pool(name="b_sd", bufs=2, space="PSUM") as b_sd,
            ):
                kp = [
                    [b_kp.tile([P, P], BF16, name=f"kp{h}_{i}") for i in range(N_RT)]
                    for h in range(H)
                ]

                def qkv_block(og):
                    aw = []
                    for ct in range(N_KC):
                        w_t = b_aw.tile(
                            [P, 512], BF16, name=f"aw{ct}_{og}", tag=f"aw{ct}", bufs=2
                        )
                        nc.sync.dma_start(
                            out=w_t[:],
                            in_=attn_wT[ct * P:(ct + 1) * P, og * 512:(og + 1) * 512],
                        )
                        aw.append(w_t)
                    for ot in range(4):
                        j = og * 4 + ot
                        ps = b_ps.tile([P, R_LOC], F32, name=f"qkvp{j}", tag="qkvp")
                        for ct in range(N_KC):
                            nc.tensor.matmul(
                                ps[:],
                                aw[ct][:, ot * P:(ot + 1) * P],
                                n1T[ct][:],
                                start=(ct == 0),
                                stop=(ct == N_KC - 1),
                            )
                        if j < H:  # k head j
                            h = j
                            rope_elu(ps, kTt[h], b_rp)
                            for i in range(N_RT):
                                nc.sync.dma_start_transpose(
                                    kp[h][i][:], kTt[h][:, i * P:(i + 1) * P]
                                )
                        elif j < 2 * H:  # v head j-16
                            h = j - H
                            vt = b_vt.tile([P, R_LOC], BF16, name=f"vT{h}", tag="vT")
                            nc.scalar.copy(vt[:], ps[:])
                            for i in range(N_RT):
                                nc.vector.memset(vp[h][i][:, HD:SB], 1.0)
                                nc.sync.dma_start_transpose(
                                    vp[h][i][:, 0:HD], vt[:, i * P:(i + 1) * P]
                                )
                        else:  # q head j-32
                            h = j - 2 * H
                            rope_elu(ps, qTt[h], b_rp)

                # k then v column groups
                for og in range(8):
                    qkv_block(og)

                # per-head local state chains + blob writes + AllGather
                for h in range(H):
                    for i in range(N_RT):
                        sd = b_sd.tile([P, SB], F32, name=f"sd{h}_{i}", tag="sd")
                        nc.tensor.matmul(
                            sd[:], kp[h][i][:], vp[h][i][:], start=True, stop=True
                        )
                        if i == 0:
                            nc.scalar.copy(e_run[h][:], sd[:])
                        else:
                            nc.vector.tensor_add(e_run[h][:], e_run[h][:], sd[:])
                        if i < 3:
                            nc.gpsimd.tensor_copy(e_st[h][i][:], e_run[h][:])
                    tb = b_stb.tile([P, SB], BF16, name=f"tb{h}", tag="tb")
                    nc.gpsimd.tensor_copy(tb[:], e_run[h][:])
                    nc.sync.dma_start(out=st_loc[:, h * SB:(h + 1) * SB], in_=tb[:])
                nc.gpsimd.collective_compute(
                    "AllGather",
                    mybir.AluOpType.bypass,
                    ins=[st_loc.ap().opt()],
                    outs=[st_all.ap().opt()],
                    replica_groups=groups,
                )

                # q column groups (overlap the collective)
                for og in range(8, 12):
                    qkv_block(og)

            # ---------------- phase C: prefix + attention ----------------
            with (
                tc.tile_pool(name="c_g", bufs=1) as c_g,
                tc.tile_pool(name="c_cmb", bufs=8) as c_cmb,
                tc.tile_pool(name="c_am", bufs=4) as c_am,
                tc.tile_pool(name="c_sm", bufs=1) as c_sm,
                tc.tile_pool(name="c_aps", bufs=2, space="PSUM") as c_aps,
                tc.tile_pool(name="c_yps", bufs=3, space="PSUM") as c_yps,
            ):
                prefix = pfx_pool.tile([P, H * SB], BF16, name="prefix")
                gtmp = pfx_pool.tile([P, H * SB], BF16, name="gtmp")
                for j in range(GROUP):
                    g_t = c_g.tile([P, H * SB], BF16, name=f"g{j}", tag="g", bufs=2)
                    nc.sync.dma_start(out=g_t[:], in_=st_all[j])
                    if j == 0:
                        nc.vector.tensor_scalar_mul(
                            prefix[:], g_t[:], pmask_sb[:, 0:1]
                        )
                    else:
                        nc.vector.tensor_scalar_mul(
                            gtmp[:], g_t[:], pmask_sb[:, j:j + 1]
                        )
                        nc.vector.tensor_add(prefix[:], prefix[:], gtmp[:])

                for i in range(N_RT):
                    isl = slice(i * P, (i + 1) * P)
                    for h in range(H):
                        hsl = slice(h * SB, (h + 1) * SB)
                        cmb = c_cmb.tile([P, SB], BF16, name=f"cmb{h}_{i}", tag="cmb")
                        if i == 0:
                            nc.gpsimd.tensor_copy(cmb[:], prefix[:, hsl])
                        else:
                            nc.vector.tensor_add(
                                cmb[:], prefix[:, hsl], e_st[h][i - 1][:]
                            )
                        a_ps = c_aps.tile([P, P], F32, name=f"a{h}_{i}", tag="a")
                        nc.tensor.matmul(
                            a_ps[:], kTt[h][:, isl], qTt[h][:, isl],
                            start=True, stop=True,
                        )
                        am = c_am.tile([P, P], BF16, name=f"am{h}_{i}", tag="am")
                        nc.vector.tensor_mul(am[:], a_ps[:], mask_sb[:])
                        y_ps = c_yps.tile([P, SB], F32, name=f"y{h}_{i}", tag="y")
                        nc.tensor.matmul(
                            y_ps[:], qTt[h][:, isl], cmb[:], start=True, stop=False
                        )
                        nc.tensor.matmul(
                            y_ps[:], am[:], vp[h][i][:], start=False, stop=True
                        )
                        rec = c_sm.tile([P, 1], F32, name=f"rec{h}_{i}", tag="rec", bufs=4)
                        nc.vector.reciprocal(rec[:], y_ps[:, HD:SB])
                        y_bf = c_sm.tile([P, HD], BF16, name=f"yb{h}_{i}", tag="yb", bufs=4)
                        nc.vector.tensor_scalar_mul(y_bf[:], y_ps[:, 0:HD], rec[:])
                        nc.sync.dma_start_transpose(yT[h][:, isl], y_bf[:])

            bc_ctx.close()  # free qkv residents for the MLP phases

            # ---------------- phase D: proj + residual + rmsnorm2 ----------
            # x2 spans D..F; n2T and gT span D..F region
            df_ctx = ExitStack()
            x2_pool = df_ctx.enter_context(tc.tile_pool(name="x2", bufs=1))
            n2T_pool = df_ctx.enter_context(tc.tile_pool(name="n2T", bufs=1))
            x2 = [x2_pool.tile([P, C], F32, name=f"x2_{i}") for i in range(N_RT)]
            n2T = [n2T_pool.tile([P, R_LOC], BF16, name=f"n2T{c}") for c in range(N_KC)]

            with (
                tc.tile_pool(name="d_pw", bufs=1) as d_pw,
                tc.tile_pool(name="d_xr", bufs=1) as d_xr,
                tc.tile_pool(name="d_st", bufs=1) as d_st,
                tc.tile_pool(name="d_nb", bufs=1) as d_nb,
                tc.tile_pool(name="d_ps", bufs=3, space="PSUM") as d_ps,
                tc.tile_pool(name="d_tps", bufs=4, space="PSUM") as d_tps,
            ):
                x_re = []
                for i in range(N_RT):
                    x_t = d_xr.tile([P, C], F32, name=f"xr{i}")
                    nc.sync.dma_start(out=x_t[:], in_=x_loc[i * P:(i + 1) * P, :])
                    x_re.append(x_t)
                for cb in range(4):
                    csl = slice(cb * 512, (cb + 1) * 512)
                    pwt = []
                    for h in range(H):
                        w_t = d_pw.tile(
                            [P, 512], BF16, name=f"pw{h}_{cb}", tag=f"pw{h}", bufs=2
                        )
                        nc.sync.dma_start(
                            out=w_t[:], in_=projwT[h * P:(h + 1) * P, csl]
                        )
                        pwt.append(w_t)
                    for rt in range(N_RT):
                        rsl = slice(rt * P, (rt + 1) * P)
                        ps = d_ps.tile([P, 512], F32, name=f"hp{rt}_{cb}", tag="hp")
                        for h in range(H):
                            nc.tensor.matmul(
                                ps[:],
                                yT[h][:, rsl],
                                pwt[h][:],
                                start=(h == 0),
                                stop=(h == H - 1),
                            )
                        nc.vector.tensor_add(x2[rt][:, csl], x_re[rt][:, csl], ps[:])
                for rt in range(N_RT):
                    sq = d_nb.tile([P, C], F32, name=f"dsq{rt}", tag="dsq", bufs=2)
                    ss = d_st.tile([P, 1], F32, name=f"dss{rt}", tag="dss", bufs=2)
                    nc.scalar.activation(sq[:], x2[rt][:], AF.Square, accum_out=ss[:])
                    rms = d_st.tile([P, 1], F32, name=f"drms{rt}", tag="drms", bufs=2)
                    nc.scalar.activation(rms[:], ss[:], AF.Sqrt, bias=eps_t[:], scale=1.0 / C)
                    inv = d_st.tile([P, 1], F32, name=f"dinv{rt}", tag="dinv", bufs=2)
                    nc.vector.reciprocal(inv[:], rms[:])
                    nb = d_nb.tile([P, C], BF16, name=f"n2b{rt}", tag="n2b", bufs=2)
                    nc.scalar.activation(nb[:], x2[rt][:], AF.Copy, scale=inv[:])
                    for j in range(N_KC):
                        tps = d_tps.tile([P, P], BF16, name=f"dtr{rt}_{j}", tag="dtr")
                        nc.tensor.transpose(tps[:], nb[:, j * P:(j + 1) * P], ident_bf[:])
                        nc.scalar.copy(n2T[j][:, rt * P:(rt + 1) * P], tps[:])

            # ---------------- phase E: fc + gelu -> gT ----------------
            g_ctx = ExitStack()
            g_pool = g_ctx.enter_context(tc.tile_pool(name="gT", bufs=1))
            gT = [g_pool.tile([P, R_LOC], BF16, name=f"g{f}") for f in range(N_FT)]
            with (
                tc.tile_pool(name="e_fw", bufs=1) as e_fw,
                tc.tile_pool(name="e_ps", bufs=3, space="PSUM") as e_ps,
            ):
                for fg in range(16):
                    fw = []
                    for ct in range(N_KC):
                        w_t = e_fw.tile(
                            [P, 512], BF16, name=f"fw{ct}_{fg}", tag=f"fw{ct}", bufs=2
                        )
                        nc.sync.dma_start(
                            out=w_t[:],
                            in_=fcwT[ct * P:(ct + 1) * P, fg * 512:(fg + 1) * 512],
                        )
                        fw.append(w_t)
                    for ft in range(4):
                        fi = fg * 4 + ft
                        ps = e_ps.tile([P, R_LOC], F32, name=f"gp{fi}", tag="gp")
                        for ct in range(N_KC):
                            nc.tensor.matmul(
                                ps[:],
                                fw[ct][:, ft * P:(ft + 1) * P],
                                n2T[ct][:],
                                start=(ct == 0),
                                stop=(ct == N_KC - 1),
                            )
                        nc.scalar.activation(gT[fi][:], ps[:], AF.Gelu)

            # ---------------- phase F: mlp_proj + residual -> out ----------
            with (
                tc.tile_pool(name="f_mw", bufs=1) as f_mw,
                tc.tile_pool(name="f_ps", bufs=1, space="PSUM") as f_ps,
                tc.tile_pool(name="f_out", bufs=4) as f_out,
            ):
                for cb in range(4):
                    csl = slice(cb * 512, (cb + 1) * 512)
                    pss = [
                        f_ps.tile(
                            [P, 512], F32, name=f"op{cb}_{rt}", tag=f"op{rt}", bufs=1
                        )
                        for rt in range(N_RT)
                    ]
                    for fi in range(N_FT):
                        mw_t = f_mw.tile(
                            [P, 512], BF16, name=f"mw{fi}_{cb}", tag=f"mw{fi}", bufs=1
                        )
                        nc.sync.dma_start(
                            out=mw_t[:], in_=mlpw[fi * P:(fi + 1) * P, csl]
                        )
                        for rt in range(N_RT):
                            nc.tensor.matmul(
                                pss[rt][:],
                                gT[fi][:, rt * P:(rt + 1) * P],
                                mw_t[:],
                                start=(fi == 0),
                                stop=(fi == N_FT - 1),
                            )
                    for rt in range(N_RT):
                        o_t = f_out.tile([P, 512], F32, name=f"o{cb}_{rt}", tag="o")
                        nc.vector.tensor_add(o_t[:], x2[rt][:, csl], pss[rt][:])
                        nc.sync.dma_start(
                            out=out_loc[rt * P:(rt + 1) * P, csl], in_=o_t[:]
                        )
            g_ctx.close()
            df_ctx.close()
            yT_ctx.close()

    return nc


_NC_CACHE = None


def _get_nc():
    global _NC_CACHE
    if _NC_CACHE is None:
        _NC_CACHE = build_nc()
    return _NC_CACHE


def _prep_inputs(x, cos, sin, attention_bias, norm1_w, norm2_w, attn_w, proj_w,
                 fc_w, mlp_proj_w):
    bf = ml_dtypes.bfloat16
    xf = np.ascontiguousarray(np.asarray(x, np.float32).reshape(R, C))
    cosTf = np.asarray(cos, np.float32).T.astype(bf)  # [HF, T]
    sinTf = np.asarray(sin, np.float32).T.astype(bf)
    # mask[s, t] = 1 iff s <= t  (transposed causal tril)
    maskT = np.triu(np.ones((P, P), np.float32))
    w1 = np.asarray(norm1_w, np.float32)
    w2 = np.asarray(norm2_w, np.float32)
    aw = (np.asarray(attn_w, np.float32) * w1[None, :]).reshape(H, 3, HD, C)
    # column order [k heads | v heads | q heads]; qkv comp order is (q,k,v)
    aw_kvq = np.concatenate([aw[:, 1], aw[:, 2], aw[:, 0]], axis=0)  # [3H, HD, C]
    attn_wTn = np.ascontiguousarray(aw_kvq.reshape(3 * C, C).T).astype(bf)
    projwTn = np.ascontiguousarray(np.asarray(proj_w, np.float32).T).astype(bf)
    fcwTn = np.ascontiguousarray(
        (np.asarray(fc_w, np.float32) * w2[None, :]).T
    ).astype(bf)
    mlpwn = np.ascontiguousarray(np.asarray(mlp_proj_w, np.float32).T).astype(bf)

    in_maps = []
    for c in range(N_CORES):
        t0 = (c * R_LOC) % T
        pm = np.zeros((P, GROUP), np.float32)
        pm[:, : c % GROUP] = 1.0
        in_maps.append({
            "x_loc": np.ascontiguousarray(xf[R_LOC * c:R_LOC * (c + 1)]),
            "cosT": np.ascontiguousarray(cosTf[:, t0:t0 + R_LOC]),
            "sinT": np.ascontiguousarray(sinTf[:, t0:t0 + R_LOC]),
            "maskT": maskT,
            "pmaskp": pm,
            "attn_wT": attn_wTn,
            "projwT": projwTn,
            "fcwT": fcwTn,
            "mlpw": mlpwn,
        })
    return in_maps


def kernel(**inputs):
    nc = _get_nc()
    in_maps = _prep_inputs(**inputs)
    res = run_bass_kernel_spmd(nc, in_maps, list(range(N_CORES)))
    out = np.concatenate(
        [np.asarray(res.results[c]["out_loc"], np.float32) for c in range(N_CORES)],
        axis=0,
    )
    return out.reshape(B, T, C)
